# revision 18
# baseline (speedup 1.0000x reference)
"""Trainium2 Bass kernel for a GPT-style causal attention block.

  y = proj( softmax_causal( (x@Wq)(x@Wk)^T / sqrt(hd) ) @ (x@Wv) )

Shapes (hardcoded): B=2, S=2048, D=1024, H=16 heads, hd=64.

Sharding over 8 NeuronCores: core = (batch b, head-group g), g selects 4
heads. Each core:
  phase 1: QKV projection for its 4 heads (bf16 inputs/weights, fp32
           psum). q,k produced TRANSPOSED [head_ch, S] (contraction-
           ready), v natural [S, head_ch] padded with 64 ONES columns:
           the AV matmul then replicates the softmax denominator across
           psum rows 64..127 for free, so normalization is a single
           psum-sourced reciprocal + multiply per head (no PE replicate,
           no psum-slot coupling with the score stream).
  phase 2: causal attention, head PAIRS in the transposed-score layout
           [key, query]: the two K=64 score matmuls of a pair run
           CONCURRENTLY in the PE array via row-group tile_position
           (0,0)/(64,0); exp on ACT (scale=1/8 folded in). Diagonal key
           tiles compute only the live query range (fully-masked columns
           are skipped by shortening the score/exp/AV moving dim), and
           the causal multiply touches only the [128,128] triangle
           block (one shared triangle tile; split DVE/gpsimd).
           Normalization is SOFTWARE-PIPELINED: block ib's chain is
           emitted after block ib+1's score/AV stream so the 3.3us DVE
           reciprocal never stalls the PE.
  phase 3: per-PAIR AllGather of aT (bf16, transposed) across the 4
           cores of the same batch: pair 0's collective+reload hides
           under pair 1's attention; the output projection contracts
           half-0 rows (already resident) while pair 1's collective
           flies, then finishes with half-1 rows. w_proj rows are
           permuted host-side to match the gathered row order.

DMA order is critical-path aware: qkv weights and the first quarter of
xT are issued first (compute starts ~8us in), constants after, the xT
tail streams under compute. Host-side sharding/layout prep is data-only
so the single SPMD program is rank-independent.
"""

import numpy as np

B = 2
S = 2048
D = 1024
H = 16
HD = 64
HLOC = 4          # heads per core
NPAIR = 2         # head pairs per core
N_CORES = 8
GROUP = 4         # cores per batch (replica group size)
IB = 512          # query block width (matmul moving dim)
JT = 128          # key tile (psum partition dim)
OC = D // GROUP   # output-projection column shard per core (256)
SCALE = 1.0 / 8.0  # 1/sqrt(hd)


def _build_bass(s=S):
    """Build the SPMD Bass program (one NeuronCore's view)."""
    import concourse.bacc as bacc
    import concourse.mybir as mybir
    import concourse.tile as tile

    f32 = mybir.dt.float32
    f32r = mybir.dt.float32r
    bf16 = mybir.dt.bfloat16
    Alu = mybir.AluOpType
    Act = mybir.ActivationFunctionType

    n_ib = s // IB           # query blocks
    n_st = s // 128          # 128-row sequence tiles
    n_dt = D // 128          # contraction tiles for D

    # Bacc (not plain Bass): its compile() lowers multi-wait sync_infos into
    # event-semaphore nops, which walrus codegen requires.
    nc = bacc.Bacc(num_devices=N_CORES)

    xt = nc.declare_dram_parameter("xt", [D, s], bf16, isOutput=False)
    wqk = nc.declare_dram_parameter("wqk", [D, 512], bf16, isOutput=False)
    wv = nc.declare_dram_parameter("wv", [D, 256], bf16, isOutput=False)
    bqk = nc.declare_dram_parameter("bqk", [128, 4], f32, isOutput=False)
    bv = nc.declare_dram_parameter("bv", [128, 256], f32, isOutput=False)
    onec = nc.declare_dram_parameter("onec", [128, 256], f32, isOutput=False)
    wp = nc.declare_dram_parameter("wp", [D, OC], bf16, isOutput=False)
    bp = nc.declare_dram_parameter("bp", [128, 2], f32, isOutput=False)
    msk = nc.declare_dram_parameter("msk", [128, 128], f32, isOutput=False)
    # y is TRANSPOSED [oc, s]: the projection then runs as 4 long psum
    # accumulation chains (moving dim 1024) instead of 128 short matmuls
    y = nc.declare_dram_parameter("y", [OC, s], f32, isOutput=True)

    with tile.TileContext(nc) as tc:
        with (
            tc.tile_pool(name="const", bufs=1) as const,
            tc.tile_pool(name="persist", bufs=1) as persist,
            tc.tile_pool(name="dram", bufs=1, space="DRAM") as dram,
        ):
            # ---- allocations (emit nothing) ----
            bqk_sb = const.tile([128, 4], f32)
            bv_sb = const.tile([128, 256], f32)
            onec_sb = const.tile([128, 256], f32)
            bp_sb = const.tile([128, 2], f32)
            msk_sb = const.tile([128, 128], f32)
            wp_sb = const.tile([128, n_dt, OC], bf16)
            warm_sb = const.tile([1, 1], f32)

            xt_sb = persist.tile([128, n_dt, s], bf16)
            wqk_sb = persist.tile([128, n_dt, 512], bf16)
            wv_sb = persist.tile([128, n_dt, 256], bf16)
            qT_sb = persist.tile([128, NPAIR, s], f32r)   # [pair_ch, pair, s]
            kT_sb = persist.tile([128, NPAIR, s], f32r)
            # [:, st, h, 0:64] = v channels, [:, st, h, 64:128] = 1.0
            v_sb = persist.tile([128, n_st, HLOC, 128], f32r)
            aT_sb = persist.tile([128, NPAIR, s], bf16)
            agf_sb = persist.tile([128, 2 * GROUP, s], bf16)  # gathered rows

            # collective buffers: [0] = pair-0 full-s, [2]/[3] = pair-1
            # s-halves (split so the first half gathers under compute)
            ag_widths = {0: s, 2: s // 2, 3: s // 2}
            ag_in = {
                i: dram.tile([128, w], bf16, name=f"ag_in{i}")
                for i, w in ag_widths.items()
            }
            ag_out = {
                i: dram.tile([GROUP * 128, w], bf16, name=f"ag_out{i}")
                for i, w in ag_widths.items()
            }

            # ---- DMA emission, critical-path order ----
            # 1. qkv weights + first quarter of xT: gates the first matmuls.
            nc.sync.dma_start(
                out=wqk_sb, in_=wqk.rearrange("(t p) c -> p t c", p=128)
            )
            nc.sync.dma_start(
                out=wv_sb, in_=wv.rearrange("(t p) c -> p t c", p=128)
            )
            # small consts the first attention block needs come before the
            # xt quarter so the v bias-add and masks are never the gate
            nc.sync.dma_start(out=bqk_sb, in_=bqk[:, :])
            nc.sync.dma_start(out=bv_sb, in_=bv[:, :])
            nc.sync.dma_start(out=onec_sb, in_=onec[:, :])
            nc.sync.dma_start(out=msk_sb, in_=msk[:, :])
            # dummy exp: pulls the ACT exp table load off the critical path
            nc.scalar.activation(
                out=warm_sb, in_=bqk_sb[0:1, 0:1], func=Act.Exp, scale=0.0
            )
            for t in range(n_dt):
                nc.sync.dma_start(
                    out=xt_sb[:, t, 0 : s // 4],
                    in_=xt.rearrange("(t p) ss -> p t ss", p=128)[:, t, 0 : s // 4],
                )
            # 2. projection consts (needed only at the tail).
            nc.sync.dma_start(out=bp_sb, in_=bp[:, :])
            nc.sync.dma_start(
                out=wp_sb, in_=wp.rearrange("(t p) c -> p t c", p=128)
            )
            # 3. rest of xT streams under compute.
            for q in range(1, 4):
                for t in range(n_dt):
                    nc.sync.dma_start(
                        out=xt_sb[:, t, q * s // 4 : (q + 1) * s // 4],
                        in_=xt.rearrange("(t p) ss -> p t ss", p=128)[
                            :, t, q * s // 4 : (q + 1) * s // 4
                        ],
                    )

            # ---- phase 1 + 2: QKV projection interleaved with attention ----
            def v_for(st_lo, st_hi, pool):
                # v natural: lhsT = xT tile [d, s-tile], rhs = Wv [d, 256]
                for st in range(st_lo, st_hi):
                    psv = pool.tile([128, 256], f32, name="psv", tag="pss")
                    for dt in range(n_dt):
                        nc.tensor.matmul(
                            psv,
                            lhsT=(xt_sb[:, dt, st * 128 : (st + 1) * 128]),
                            rhs=(wv_sb[:, dt, :]),
                            start=(dt == 0),
                            stop=(dt == n_dt - 1),
                        )
                    nc.vector.tensor_tensor(
                        out=v_sb[:, st, :, 0:64],
                        in0=psv.rearrange("p (h e) -> p h e", h=HLOC),
                        in1=bv_sb.rearrange("p (h e) -> p h e", h=HLOC),
                        op=Alu.add,
                    )
                    # ones columns 64..127: AV replicates the softmax
                    # denominator across psum rows 64..127
                    nc.vector.tensor_copy(
                        out=v_sb[:, st, :, 64:128],
                        in_=onec_sb.rearrange("p (h e) -> p h e", h=HLOC),
                    )

            def qkT_sb(t, sb, pool):
                # qT/kT: lhsT = W tile [d,c], rhs = xT [d, s-block]
                # c-tile t: 0,1 = q pair0/1; 2,3 = k pair0/1
                ps = pool.tile([128, IB], f32, name="ps", tag="pss")
                for dt in range(n_dt):
                    nc.tensor.matmul(
                        ps,
                        lhsT=(wqk_sb[:, dt, t * 128 : (t + 1) * 128]),
                        rhs=(xt_sb[:, dt, sb * IB : (sb + 1) * IB]),
                        start=(dt == 0),
                        stop=(dt == n_dt - 1),
                    )
                dst = qT_sb if t < 2 else kT_sb
                nc.vector.tensor_scalar_add(
                    out=dst[:, t % 2, sb * IB : (sb + 1) * IB],
                    in0=ps,
                    scalar1=bqk_sb[:, t : t + 1],
                )

            with (
                tc.tile_pool(name="ps_s", bufs=2, space="PSUM") as ps_s,
                tc.tile_pool(name="ps_av", bufs=2, space="PSUM") as ps_av,
                tc.tile_pool(name="pt", bufs=4) as ptpool,
                tc.tile_pool(name="small", bufs=4) as small,
            ):

                def scores_av_emit(pair, ib):
                    """Scores + exp + causal mask + AV for query block ib.
                    Returns the two [128,IB] psum accumulators (rows 64..127
                    = replicated softmax denominator). Diagonal key tile k
                    only touches query columns >= 128k (the rest are fully
                    masked), which shortens its score/exp/AV moving dim."""
                    njt = 4 * (ib + 1)  # key tiles needed (j <= i)
                    avs = [
                        ps_av.tile([128, IB], f32, name=f"av{hh}", tag=f"av{hh}")
                        for hh in range(2)
                    ]
                    # diagonal key tiles first: their mask multiply then
                    # overlaps the long non-diagonal score/AV stream
                    jt_order = list(range(4 * ib, njt)) + list(range(4 * ib))
                    for jseq, jt in enumerate(jt_order):
                        k = jt - 4 * ib  # >= 0: diagonal tile index
                        lo = 128 * k if k > 0 else 0  # first live query col
                        pss = ps_s.tile([128, 2 * IB], f32, name="pss", tag="pss")
                        for hh in range(2):
                            off = hh * 64
                            nc.tensor.matmul(
                                pss[:, hh * IB + lo : (hh + 1) * IB],
                                lhsT=(kT_sb[
                                        off : off + 64,
                                        pair,
                                        jt * 128 : (jt + 1) * 128,
                                    ]
                                ),
                                rhs=(qT_sb[
                                        off : off + 64,
                                        pair,
                                        ib * IB + lo : (ib + 1) * IB,
                                    ]
                                ),
                                start=True,
                                stop=True,
                                tile_position=(off, 0),
                            )
                        pt = ptpool.tile([128, 2, IB], f32r, name="pt")
                        nc.scalar.activation(
                            out=pt[:, :, lo:IB],
                            in_=pss.rearrange("p (h q) -> p h q", h=2)[
                                :, :, lo:IB
                            ],
                            func=Act.Exp,
                            scale=SCALE,
                        )
                        for hh in range(2):
                            if k >= 0:  # causal triangle on the k-th block
                                eng = nc.vector if hh == 0 else nc.gpsimd
                                eng.tensor_tensor(
                                    out=pt[:, hh, lo : lo + 128],
                                    in0=pt[:, hh, lo : lo + 128],
                                    in1=msk_sb,
                                    op=Alu.mult,
                                )
                            nc.tensor.matmul(
                                avs[hh][:, lo:IB],
                                lhsT=(v_sb[:, jt, pair * 2 + hh, :]),
                                rhs=(pt[:, hh, lo:IB]),
                                start=(jseq == 0),
                                stop=(jseq == njt - 1),
                            )
                    return avs

                def normalize_emit(pair, ib, avs):
                    """aT[:, ib block] = av[0:64] * (1 / av[64:128]): the
                    denominator arrives already replicated from the ones
                    columns of v, so this is one fast-approx reciprocal
                    (~18 bits, 5x faster than exact) + one multiply per
                    head, both DVE, no PE involvement."""
                    for hh in range(2):
                        zsb = small.tile([64, IB], f32, name="zsb")
                        nc.vector.tensor_copy(out=zsb, in_=avs[hh][64:128, :])
                        recr = small.tile([64, IB], f32, name="recr")
                        nc.vector.reciprocal_approx_fast(out=recr, in_=zsb)
                        nc.vector.tensor_tensor(
                            out=aT_sb[
                                hh * 64 : (hh + 1) * 64,
                                pair,
                                ib * IB : (ib + 1) * IB,
                            ],
                            in0=recr,
                            in1=avs[hh][0:64, :],
                            op=Alu.mult,
                        )

                def gather_emit(pair, half):
                    """Stage + AllGather + reload one s-half of a pair's aT.
                    half=None gathers the full s range in one collective."""
                    lo, hi = (0, s) if half is None else (
                        half * s // 2, (half + 1) * s // 2
                    )
                    idx = 2 * pair + (half or 0)
                    gin, gout = ag_in[idx], ag_out[idx]
                    nc.sync.dma_start(out=gin, in_=aT_sb[:, pair, lo:hi])
                    nc.gpsimd.collective_compute(
                        "AllGather",
                        Alu.bypass,
                        replica_groups=[[0, 1, 2, 3], [4, 5, 6, 7]],
                        ins=[gin[:, :]],
                        outs=[gout[:, :]],
                    )
                    for t in range(GROUP):
                        nc.sync.dma_start(
                            out=agf_sb[:, pair * GROUP + t, lo:hi],
                            in_=gout.rearrange("(t p) ss -> p t ss", p=128)[
                                :, t, :
                            ],
                        )

                for pair in range(NPAIR):
                    pending = None
                    for ib in range(n_ib):
                        # just-in-time qT/kT for this query block: the first
                        # score matmuls start after one xT quarter arrives
                        qkT_sb(pair, ib, ps_s)
                        qkT_sb(2 + pair, ib, ps_s)
                        if pair == 0:
                            # v quarter-blocks on demand: attention for this
                            # ib only needs v key-tiles st <= 4*ib+3
                            v_for(4 * ib, 4 * ib + 4, ps_s)
                        avs = scores_av_emit(pair, ib)
                        # deferred normalize: ib-1's reciprocal chain hides
                        # under ib's score/AV stream on the PE
                        if pending is not None:
                            normalize_emit(pair, ib - 1, pending)
                        pending = avs
                        if pair == 1 and ib == 2:
                            # aT[:, 1, 0:s/2] (ibs 0-1) is complete: gather
                            # the first half under ibs 2-3's compute
                            gather_emit(1, 0)
                    normalize_emit(pair, n_ib - 1, pending)
                    if pair == 0:
                        # single collective: fully hidden under pair 1
                        gather_emit(0, None)
                    else:
                        gather_emit(1, 1)

            # ---- phase 3: output projection, transposed + split by s ----
            # yT[oc, s] = wp^T @ a_gathered: 4 psum chains of 8 matmuls with
            # moving dim s/2 = 1024 (dense, p-state friendly). The s-half 0
            # chains need only AG0 + AG1a and run while AG1b flies.
            with (
                tc.tile_pool(name="ps_y", bufs=4, space="PSUM") as ps_y,
                tc.tile_pool(name="yout", bufs=3) as yout,
            ):
                for sq in range(4):  # s quarters (psum bank = 512 f32)
                    lo, hi = sq * s // 4, (sq + 1) * s // 4
                    yts = {}
                    for ocb in range(2):
                        yts[ocb] = ps_y.tile([128, s // 4], f32, name="yt")
                    for t in range(2 * GROUP):
                        for ocb in range(2):
                            nc.tensor.matmul(
                                yts[ocb],
                                lhsT=(wp_sb[:, t, ocb * 128 : (ocb + 1) * 128]),
                                rhs=(agf_sb[:, t, lo:hi]),
                                start=(t == 0),
                                stop=(t == 2 * GROUP - 1),
                            )
                    for ocb in range(2):
                        ysb = yout.tile([128, s // 4], f32, name="ysb")
                        nc.vector.tensor_scalar_add(
                            out=ysb,
                            in0=yts[ocb],
                            scalar1=bp_sb[:, ocb : ocb + 1],
                        )
                        nc.sync.dma_start(
                            out=y[ocb * 128 : (ocb + 1) * 128, lo:hi],
                            in_=ysb,
                        )

    nc.compile()
    return nc


def _shard_inputs(x, w_attn, b_attn, w_proj, b_proj, s=S):
    """Host-side sharding: build the per-core input maps."""
    import ml_dtypes

    bfl = ml_dtypes.bfloat16
    x = np.asarray(x, dtype=np.float32)
    w_attn = np.asarray(w_attn, dtype=np.float32)
    b_attn = np.asarray(b_attn, dtype=np.float32)
    w_proj = np.asarray(w_proj, dtype=np.float32)
    b_proj = np.asarray(b_proj, dtype=np.float32)

    # causal triangle tile: msk[j, i] = 1.0 if i >= j (shared by every
    # diagonal key tile after its fully-masked columns are skipped)
    msk = (np.arange(128)[None, :] >= np.arange(128)[:, None]).astype(
        np.float32
    )

    # w_proj row permutation matching the two per-pair gathers:
    # half 0 = [core0 pair0 (128 rows), core1 pair0, ...], half 1 = pair1.
    perm = np.concatenate(
        [np.arange(c * 256, c * 256 + 128) for c in range(GROUP)]
        + [np.arange(c * 256 + 128, (c + 1) * 256) for c in range(GROUP)]
    )

    in_maps = []
    for core in range(N_CORES):
        b, g = divmod(core, GROUP)
        hs = list(range(g * HLOC, (g + 1) * HLOC))
        xt = np.ascontiguousarray(x[b].T).astype(bfl)
        qcols = np.concatenate(
            [w_attn[:, h * HD : (h + 1) * HD] for h in hs], axis=1
        )
        kcols = np.concatenate(
            [w_attn[:, D + h * HD : D + (h + 1) * HD] for h in hs], axis=1
        )
        vcols = np.concatenate(
            [w_attn[:, 2 * D + h * HD : 2 * D + (h + 1) * HD] for h in hs],
            axis=1,
        )
        wqk = np.concatenate([qcols, kcols], axis=1).astype(bfl)
        bq = np.concatenate([b_attn[h * HD : (h + 1) * HD] for h in hs])
        bk = np.concatenate([b_attn[D + h * HD : D + (h + 1) * HD] for h in hs])
        bvv = np.concatenate(
            [b_attn[2 * D + h * HD : 2 * D + (h + 1) * HD] for h in hs]
        )
        bqk = np.concatenate([bq, bk]).reshape(4, 128).T.copy()  # [128, 4]
        bv = np.broadcast_to(bvv, (128, 256)).copy()
        wpc = np.ascontiguousarray(
            w_proj[perm][:, g * OC : (g + 1) * OC]
        ).astype(bfl)
        # transposed-projection bias: column ocb holds b_proj for oc rows
        # [ocb*128, (ocb+1)*128) of this core's shard
        bpc = b_proj[g * OC : (g + 1) * OC].reshape(2, 128).T.copy()
        in_maps.append(
            dict(
                xt=xt, wqk=wqk, wv=vcols.astype(bfl), bqk=bqk, bv=bv,
                onec=np.ones((128, 256), np.float32), wp=wpc, bp=bpc, msk=msk,
            )
        )
    return in_maps


def _unshard(results):
    y = np.empty((B, S, D), np.float32)
    for core in range(N_CORES):
        b, g = divmod(core, GROUP)
        y[b, :, g * OC : (g + 1) * OC] = results[core]["y"].T
    return y


_NC_CACHE = {}


def kernel(x, w_attn, b_attn, w_proj, b_proj):
    from concourse.bass_utils import run_bass_kernel_spmd

    if S not in _NC_CACHE:
        _NC_CACHE[S] = _build_bass(S)
    nc = _NC_CACHE[S]
    in_maps = _shard_inputs(x, w_attn, b_attn, w_proj, b_proj)
    res = run_bass_kernel_spmd(nc, in_maps, list(range(N_CORES)))
    return _unshard(res.results)


# revision 20
# speedup vs baseline: 1.1115x; 1.1115x over previous
"""Trainium2 Bass kernel for a GPT-style causal attention block.

  y = proj( softmax_causal( (x@Wq)(x@Wk)^T / sqrt(hd) ) @ (x@Wv) )

Shapes (hardcoded): B=2, S=2048, D=1024, H=16 heads, hd=64.

Sharding over 8 NeuronCores: core = (batch b, head-group g), g selects 4
heads. Each core:
  phase 1: QKV projection for its 4 heads (bf16 inputs/weights, fp32
           psum). q,k produced TRANSPOSED [head_ch, S] (contraction-
           ready), v natural [S, head_ch] padded with 64 ONES columns:
           the AV matmul then replicates the softmax denominator across
           psum rows 64..127 for free, so normalization is a single
           psum-sourced reciprocal + multiply per head (no PE replicate,
           no psum-slot coupling with the score stream).
  phase 2: causal attention, head PAIRS in the transposed-score layout
           [key, query]: the two K=64 score matmuls of a pair run
           CONCURRENTLY in the PE array via row-group tile_position
           (0,0)/(64,0); exp on ACT (scale=1/8 folded in). Diagonal key
           tiles compute only the live query range (fully-masked columns
           are skipped by shortening the score/exp/AV moving dim), and
           the causal multiply touches only the [128,128] triangle
           block (one shared triangle tile; split DVE/gpsimd).
           Normalization is SOFTWARE-PIPELINED: block ib's chain is
           emitted after block ib+1's score/AV stream so the 3.3us DVE
           reciprocal never stalls the PE.
  phase 3: per-PAIR AllGather of aT (bf16, transposed) across the 4
           cores of the same batch: pair 0's collective+reload hides
           under pair 1's attention; the output projection contracts
           half-0 rows (already resident) while pair 1's collective
           flies, then finishes with half-1 rows. w_proj rows are
           permuted host-side to match the gathered row order.

DMA order is critical-path aware: qkv weights and the first quarter of
xT are issued first (compute starts ~8us in), constants after, the xT
tail streams under compute. Host-side sharding/layout prep is data-only
so the single SPMD program is rank-independent.
"""

import numpy as np

B = 2
S = 2048
D = 1024
H = 16
HD = 64
HLOC = 4          # heads per core
NPAIR = 2         # head pairs per core
N_CORES = 8
GROUP = 4         # cores per batch (replica group size)
IB = 512          # query block width (matmul moving dim)
JT = 128          # key tile (psum partition dim)
OC = D // GROUP   # output-projection column shard per core (256)
SCALE = 1.0 / 8.0  # 1/sqrt(hd)


def _build_bass(s=S):
    """Build the SPMD Bass program (one NeuronCore's view)."""
    import concourse.bacc as bacc
    import concourse.mybir as mybir
    import concourse.tile as tile

    f32 = mybir.dt.float32
    f32r = mybir.dt.float32r
    bf16 = mybir.dt.bfloat16
    Alu = mybir.AluOpType
    Act = mybir.ActivationFunctionType

    n_ib = s // IB           # query blocks
    n_st = s // 128          # 128-row sequence tiles
    n_dt = D // 128          # contraction tiles for D

    # Bacc (not plain Bass): its compile() lowers multi-wait sync_infos into
    # event-semaphore nops, which walrus codegen requires.
    nc = bacc.Bacc(num_devices=N_CORES)

    xt = nc.declare_dram_parameter("xt", [D, s], bf16, isOutput=False)
    wqk = nc.declare_dram_parameter("wqk", [D, 512], bf16, isOutput=False)
    wv = nc.declare_dram_parameter("wv", [D, 256], bf16, isOutput=False)
    bqk = nc.declare_dram_parameter("bqk", [128, 4], f32, isOutput=False)
    bv = nc.declare_dram_parameter("bv", [128, 256], f32, isOutput=False)
    onec = nc.declare_dram_parameter("onec", [128, 256], f32, isOutput=False)
    wp = nc.declare_dram_parameter("wp", [D, OC], bf16, isOutput=False)
    bp = nc.declare_dram_parameter("bp", [128, 2], f32, isOutput=False)
    msk = nc.declare_dram_parameter("msk", [128, 128], f32, isOutput=False)
    # y is TRANSPOSED [oc, s]: the projection then runs as 4 long psum
    # accumulation chains (moving dim 1024) instead of 128 short matmuls
    y = nc.declare_dram_parameter("y", [OC, s], f32, isOutput=True)

    with tile.TileContext(nc) as tc:
        with (
            tc.tile_pool(name="const", bufs=1) as const,
            tc.tile_pool(name="persist", bufs=1) as persist,
            tc.tile_pool(name="dram", bufs=1, space="DRAM") as dram,
        ):
            # ---- allocations (emit nothing) ----
            bqk_sb = const.tile([128, 4], f32)
            bv_sb = const.tile([128, 256], f32)
            onec_sb = const.tile([128, 256], f32)
            bp_sb = const.tile([128, 2], f32)
            msk_sb = const.tile([128, 128], f32)
            wp_sb = const.tile([128, n_dt, OC], bf16)
            warm_sb = const.tile([1, 1], f32)

            xt_sb = persist.tile([128, n_dt, s], bf16)
            wqk_sb = persist.tile([128, n_dt, 512], bf16)
            wv_sb = persist.tile([128, n_dt, 256], bf16)
            qT_sb = persist.tile([128, NPAIR, s], f32r)   # [pair_ch, pair, s]
            kT_sb = persist.tile([128, NPAIR, s], f32r)
            # [:, st, h, 0:64] = v channels, [:, st, h, 64:128] = 1.0
            v_sb = persist.tile([128, n_st, HLOC, 128], f32r)
            aT_sb = persist.tile([128, NPAIR, s], bf16)
            agf_sb = persist.tile([128, 2 * GROUP, s], bf16)  # gathered rows

            # collective buffers: [0] = pair-0 full-s, [2]/[3] = pair-1
            # s-halves (split so the first half gathers under compute)
            ag_widths = {0: s, 2: s // 2, 3: s // 2}
            ag_in = {
                i: dram.tile([128, w], bf16, name=f"ag_in{i}")
                for i, w in ag_widths.items()
            }
            ag_out = {
                i: dram.tile([GROUP * 128, w], bf16, name=f"ag_out{i}")
                for i, w in ag_widths.items()
            }

            # ---- DMA emission, critical-path order ----
            # 1. qkv weights + first quarter of xT: gates the first matmuls.
            nc.sync.dma_start(
                out=wqk_sb, in_=wqk.rearrange("(t p) c -> p t c", p=128)
            )
            nc.sync.dma_start(
                out=wv_sb, in_=wv.rearrange("(t p) c -> p t c", p=128)
            )
            # one trigger per xt quarter: each dma_start costs ~0.7us on the
            # serial Sync queue, so chunk count — not bytes — gated startup
            nc.sync.dma_start(
                out=xt_sb[:, :, 0 : s // 4],
                in_=xt.rearrange("(t p) ss -> p t ss", p=128)[:, :, 0 : s // 4],
            )
            # 2. small consts the first attention block needs.
            nc.sync.dma_start(out=bqk_sb, in_=bqk[:, :])
            nc.sync.dma_start(out=bv_sb, in_=bv[:, :])
            nc.sync.dma_start(out=onec_sb, in_=onec[:, :])
            nc.sync.dma_start(out=msk_sb, in_=msk[:, :])
            # dummy exp: pulls the ACT exp table load off the critical path
            nc.scalar.activation(
                out=warm_sb, in_=bqk_sb[0:1, 0:1], func=Act.Exp, scale=0.0
            )
            # 3. rest of xT + projection consts stream under compute.
            for q in range(1, 4):
                nc.sync.dma_start(
                    out=xt_sb[:, :, q * s // 4 : (q + 1) * s // 4],
                    in_=xt.rearrange("(t p) ss -> p t ss", p=128)[
                        :, :, q * s // 4 : (q + 1) * s // 4
                    ],
                )
            nc.sync.dma_start(out=bp_sb, in_=bp[:, :])
            nc.sync.dma_start(
                out=wp_sb, in_=wp.rearrange("(t p) c -> p t c", p=128)
            )

            # ---- phase 1 + 2: QKV projection interleaved with attention ----
            def v_for(st_lo, st_hi, pool):
                # v natural: lhsT = xT tile [d, s-tile], rhs = Wv [d, 256]
                for st in range(st_lo, st_hi):
                    psv = pool.tile([128, 256], f32, name="psv", tag="pss")
                    for dt in range(n_dt):
                        nc.tensor.matmul(
                            psv,
                            lhsT=(xt_sb[:, dt, st * 128 : (st + 1) * 128]),
                            rhs=(wv_sb[:, dt, :]),
                            start=(dt == 0),
                            stop=(dt == n_dt - 1),
                        )
                    nc.vector.tensor_tensor(
                        out=v_sb[:, st, :, 0:64],
                        in0=psv.rearrange("p (h e) -> p h e", h=HLOC),
                        in1=bv_sb.rearrange("p (h e) -> p h e", h=HLOC),
                        op=Alu.add,
                    )
                    # ones columns 64..127: AV replicates the softmax
                    # denominator across psum rows 64..127
                    nc.vector.tensor_copy(
                        out=v_sb[:, st, :, 64:128],
                        in_=onec_sb.rearrange("p (h e) -> p h e", h=HLOC),
                    )

            def qkT_sb(t, sb, pool):
                # qT/kT: lhsT = W tile [d,c], rhs = xT [d, s-block]
                # c-tile t: 0,1 = q pair0/1; 2,3 = k pair0/1
                ps = pool.tile([128, IB], f32, name="ps", tag="pss")
                for dt in range(n_dt):
                    nc.tensor.matmul(
                        ps,
                        lhsT=(wqk_sb[:, dt, t * 128 : (t + 1) * 128]),
                        rhs=(xt_sb[:, dt, sb * IB : (sb + 1) * IB]),
                        start=(dt == 0),
                        stop=(dt == n_dt - 1),
                    )
                dst = qT_sb if t < 2 else kT_sb
                nc.vector.tensor_scalar_add(
                    out=dst[:, t % 2, sb * IB : (sb + 1) * IB],
                    in0=ps,
                    scalar1=bqk_sb[:, t : t + 1],
                )

            with (
                tc.tile_pool(name="ps_s", bufs=2, space="PSUM") as ps_s,
                tc.tile_pool(name="ps_av", bufs=2, space="PSUM") as ps_av,
                tc.tile_pool(name="pt", bufs=4) as ptpool,
                tc.tile_pool(name="small", bufs=4) as small,
            ):

                def scores_av_emit(pair, ib):
                    """Scores + exp + causal mask + AV for query block ib.
                    Returns the two [128,IB] psum accumulators (rows 64..127
                    = replicated softmax denominator). Diagonal key tile k
                    only touches query columns >= 128k (the rest are fully
                    masked), which shortens its score/exp/AV moving dim."""
                    njt = 4 * (ib + 1)  # key tiles needed (j <= i)
                    avs = [
                        ps_av.tile([128, IB], f32, name=f"av{hh}", tag=f"av{hh}")
                        for hh in range(2)
                    ]
                    # diagonal key tiles first: their mask multiply then
                    # overlaps the long non-diagonal score/AV stream
                    jt_order = list(range(4 * ib, njt)) + list(range(4 * ib))
                    for jseq, jt in enumerate(jt_order):
                        k = jt - 4 * ib  # >= 0: diagonal tile index
                        lo = 128 * k if k > 0 else 0  # first live query col
                        pss = ps_s.tile([128, 2 * IB], f32, name="pss", tag="pss")
                        for hh in range(2):
                            off = hh * 64
                            nc.tensor.matmul(
                                pss[:, hh * IB + lo : (hh + 1) * IB],
                                lhsT=(kT_sb[
                                        off : off + 64,
                                        pair,
                                        jt * 128 : (jt + 1) * 128,
                                    ]
                                ),
                                rhs=(qT_sb[
                                        off : off + 64,
                                        pair,
                                        ib * IB + lo : (ib + 1) * IB,
                                    ]
                                ),
                                start=True,
                                stop=True,
                                tile_position=(off, 0),
                            )
                        pt = ptpool.tile([128, 2, IB], f32r, name="pt")
                        nc.scalar.activation(
                            out=pt[:, :, lo:IB],
                            in_=pss.rearrange("p (h q) -> p h q", h=2)[
                                :, :, lo:IB
                            ],
                            func=Act.Exp,
                            scale=SCALE,
                        )
                        for hh in range(2):
                            if k >= 0:  # causal triangle on the k-th block
                                eng = nc.vector if hh == 0 else nc.gpsimd
                                eng.tensor_tensor(
                                    out=pt[:, hh, lo : lo + 128],
                                    in0=pt[:, hh, lo : lo + 128],
                                    in1=msk_sb,
                                    op=Alu.mult,
                                )
                            nc.tensor.matmul(
                                avs[hh][:, lo:IB],
                                lhsT=(v_sb[:, jt, pair * 2 + hh, :]),
                                rhs=(pt[:, hh, lo:IB]),
                                start=(jseq == 0),
                                stop=(jseq == njt - 1),
                            )
                    return avs

                def normalize_emit(pair, ib, avs):
                    """aT[:, ib block] = av[0:64] * (1 / av[64:128]): the
                    denominator arrives already replicated from the ones
                    columns of v, so this is one fast-approx reciprocal
                    (~18 bits, 5x faster than exact) + one multiply per
                    head, both DVE, no PE involvement."""
                    for hh in range(2):
                        zsb = small.tile([64, IB], f32, name="zsb")
                        nc.vector.tensor_copy(out=zsb, in_=avs[hh][64:128, :])
                        recr = small.tile([64, IB], f32, name="recr")
                        nc.vector.reciprocal_approx_fast(out=recr, in_=zsb)
                        nc.vector.tensor_tensor(
                            out=aT_sb[
                                hh * 64 : (hh + 1) * 64,
                                pair,
                                ib * IB : (ib + 1) * IB,
                            ],
                            in0=recr,
                            in1=avs[hh][0:64, :],
                            op=Alu.mult,
                        )

                def gather_emit(pair, half):
                    """Stage + AllGather + reload one s-half of a pair's aT.
                    half=None gathers the full s range in one collective."""
                    lo, hi = (0, s) if half is None else (
                        half * s // 2, (half + 1) * s // 2
                    )
                    idx = 2 * pair + (half or 0)
                    gin, gout = ag_in[idx], ag_out[idx]
                    nc.sync.dma_start(out=gin, in_=aT_sb[:, pair, lo:hi])
                    nc.gpsimd.collective_compute(
                        "AllGather",
                        Alu.bypass,
                        replica_groups=[[0, 1, 2, 3], [4, 5, 6, 7]],
                        ins=[gin[:, :]],
                        outs=[gout[:, :]],
                    )
                    # reload by s-quarter (all 4 gathered row-blocks per
                    # trigger): the first projection quarter that needs this
                    # gather can start after ~1MB instead of the full reload
                    w = hi - lo
                    for q in range(2):
                        nc.sync.dma_start(
                            out=agf_sb[
                                :,
                                pair * GROUP : (pair + 1) * GROUP,
                                lo + q * w // 2 : lo + (q + 1) * w // 2,
                            ],
                            in_=gout.rearrange("(t p) ss -> p t ss", p=128)[
                                :, :, q * w // 2 : (q + 1) * w // 2
                            ],
                        )

                for pair in range(NPAIR):
                    pending = None
                    for ib in range(n_ib):
                        # just-in-time qT/kT for this query block: the first
                        # score matmuls start after one xT quarter arrives
                        qkT_sb(pair, ib, ps_s)
                        qkT_sb(2 + pair, ib, ps_s)
                        if pair == 0:
                            # v quarter-blocks on demand: attention for this
                            # ib only needs v key-tiles st <= 4*ib+3
                            v_for(4 * ib, 4 * ib + 4, ps_s)
                        avs = scores_av_emit(pair, ib)
                        # deferred normalize: ib-1's reciprocal chain hides
                        # under ib's score/AV stream on the PE
                        if pending is not None:
                            normalize_emit(pair, ib - 1, pending)
                        pending = avs
                        if pair == 1 and ib == 2:
                            # aT[:, 1, 0:s/2] (ibs 0-1) is complete: gather
                            # the first half under ibs 2-3's compute
                            gather_emit(1, 0)
                    normalize_emit(pair, n_ib - 1, pending)
                    if pair == 0:
                        # single collective: fully hidden under pair 1
                        gather_emit(0, None)
                    else:
                        gather_emit(1, 1)

            # ---- phase 3: output projection, transposed + split by s ----
            # yT[oc, s] = wp^T @ a_gathered: 4 psum chains of 8 matmuls with
            # moving dim s/2 = 1024 (dense, p-state friendly). The s-half 0
            # chains need only AG0 + AG1a and run while AG1b flies.
            with (
                tc.tile_pool(name="ps_y", bufs=4, space="PSUM") as ps_y,
                tc.tile_pool(name="yout", bufs=3) as yout,
            ):
                for sq in range(4):  # s quarters (psum bank = 512 f32)
                    lo, hi = sq * s // 4, (sq + 1) * s // 4
                    yts = {}
                    for ocb in range(2):
                        yts[ocb] = ps_y.tile([128, s // 4], f32, name="yt")
                    for t in range(2 * GROUP):
                        for ocb in range(2):
                            nc.tensor.matmul(
                                yts[ocb],
                                lhsT=(wp_sb[:, t, ocb * 128 : (ocb + 1) * 128]),
                                rhs=(agf_sb[:, t, lo:hi]),
                                start=(t == 0),
                                stop=(t == 2 * GROUP - 1),
                            )
                    for ocb in range(2):
                        ysb = yout.tile([128, s // 4], f32, name="ysb")
                        nc.vector.tensor_scalar_add(
                            out=ysb,
                            in0=yts[ocb],
                            scalar1=bp_sb[:, ocb : ocb + 1],
                        )
                        nc.sync.dma_start(
                            out=y[ocb * 128 : (ocb + 1) * 128, lo:hi],
                            in_=ysb,
                        )

    nc.compile()
    return nc


def _shard_inputs(x, w_attn, b_attn, w_proj, b_proj, s=S):
    """Host-side sharding: build the per-core input maps."""
    import ml_dtypes

    bfl = ml_dtypes.bfloat16
    x = np.asarray(x, dtype=np.float32)
    w_attn = np.asarray(w_attn, dtype=np.float32)
    b_attn = np.asarray(b_attn, dtype=np.float32)
    w_proj = np.asarray(w_proj, dtype=np.float32)
    b_proj = np.asarray(b_proj, dtype=np.float32)

    # causal triangle tile: msk[j, i] = 1.0 if i >= j (shared by every
    # diagonal key tile after its fully-masked columns are skipped)
    msk = (np.arange(128)[None, :] >= np.arange(128)[:, None]).astype(
        np.float32
    )

    # w_proj row permutation matching the two per-pair gathers:
    # half 0 = [core0 pair0 (128 rows), core1 pair0, ...], half 1 = pair1.
    perm = np.concatenate(
        [np.arange(c * 256, c * 256 + 128) for c in range(GROUP)]
        + [np.arange(c * 256 + 128, (c + 1) * 256) for c in range(GROUP)]
    )

    in_maps = []
    for core in range(N_CORES):
        b, g = divmod(core, GROUP)
        hs = list(range(g * HLOC, (g + 1) * HLOC))
        xt = np.ascontiguousarray(x[b].T).astype(bfl)
        qcols = np.concatenate(
            [w_attn[:, h * HD : (h + 1) * HD] for h in hs], axis=1
        )
        kcols = np.concatenate(
            [w_attn[:, D + h * HD : D + (h + 1) * HD] for h in hs], axis=1
        )
        vcols = np.concatenate(
            [w_attn[:, 2 * D + h * HD : 2 * D + (h + 1) * HD] for h in hs],
            axis=1,
        )
        wqk = np.concatenate([qcols, kcols], axis=1).astype(bfl)
        bq = np.concatenate([b_attn[h * HD : (h + 1) * HD] for h in hs])
        bk = np.concatenate([b_attn[D + h * HD : D + (h + 1) * HD] for h in hs])
        bvv = np.concatenate(
            [b_attn[2 * D + h * HD : 2 * D + (h + 1) * HD] for h in hs]
        )
        bqk = np.concatenate([bq, bk]).reshape(4, 128).T.copy()  # [128, 4]
        bv = np.broadcast_to(bvv, (128, 256)).copy()
        wpc = np.ascontiguousarray(
            w_proj[perm][:, g * OC : (g + 1) * OC]
        ).astype(bfl)
        # transposed-projection bias: column ocb holds b_proj for oc rows
        # [ocb*128, (ocb+1)*128) of this core's shard
        bpc = b_proj[g * OC : (g + 1) * OC].reshape(2, 128).T.copy()
        in_maps.append(
            dict(
                xt=xt, wqk=wqk, wv=vcols.astype(bfl), bqk=bqk, bv=bv,
                onec=np.ones((128, 256), np.float32), wp=wpc, bp=bpc, msk=msk,
            )
        )
    return in_maps


def _unshard(results):
    y = np.empty((B, S, D), np.float32)
    for core in range(N_CORES):
        b, g = divmod(core, GROUP)
        y[b, :, g * OC : (g + 1) * OC] = results[core]["y"].T
    return y


_NC_CACHE = {}


def kernel(x, w_attn, b_attn, w_proj, b_proj):
    from concourse.bass_utils import run_bass_kernel_spmd

    if S not in _NC_CACHE:
        _NC_CACHE[S] = _build_bass(S)
    nc = _NC_CACHE[S]
    in_maps = _shard_inputs(x, w_attn, b_attn, w_proj, b_proj)
    res = run_bass_kernel_spmd(nc, in_maps, list(range(N_CORES)))
    return _unshard(res.results)


# revision 21
# speedup vs baseline: 1.1403x; 1.0259x over previous
"""Trainium2 Bass kernel for a GPT-style causal attention block.

  y = proj( softmax_causal( (x@Wq)(x@Wk)^T / sqrt(hd) ) @ (x@Wv) )

Shapes (hardcoded): B=2, S=2048, D=1024, H=16 heads, hd=64.

Sharding over 8 NeuronCores: core = (batch b, head-group g), g selects 4
heads. Each core:
  phase 1: QKV projection for its 4 heads (bf16 inputs/weights, fp32
           psum). q,k produced TRANSPOSED [head_ch, S] (contraction-
           ready), v natural [S, head_ch] padded with 64 ONES columns:
           the AV matmul then replicates the softmax denominator across
           psum rows 64..127 for free, so normalization is a single
           psum-sourced reciprocal + multiply per head (no PE replicate,
           no psum-slot coupling with the score stream).
  phase 2: causal attention, head PAIRS in the transposed-score layout
           [key, query]: the two K=64 score matmuls of a pair run
           CONCURRENTLY in the PE array via row-group tile_position
           (0,0)/(64,0); exp on ACT (scale=1/8 folded in). Diagonal key
           tiles compute only the live query range (fully-masked columns
           are skipped by shortening the score/exp/AV moving dim), and
           the causal multiply touches only the [128,128] triangle
           block (one shared triangle tile; split DVE/gpsimd).
           Normalization is SOFTWARE-PIPELINED: block ib's chain is
           emitted after block ib+1's score/AV stream so the 3.3us DVE
           reciprocal never stalls the PE.
  phase 3: per-PAIR AllGather of aT (bf16, transposed) across the 4
           cores of the same batch: pair 0's collective+reload hides
           under pair 1's attention; the output projection contracts
           half-0 rows (already resident) while pair 1's collective
           flies, then finishes with half-1 rows. w_proj rows are
           permuted host-side to match the gathered row order.

DMA order is critical-path aware: qkv weights and the first quarter of
xT are issued first (compute starts ~8us in), constants after, the xT
tail streams under compute. Host-side sharding/layout prep is data-only
so the single SPMD program is rank-independent.
"""

import numpy as np

B = 2
S = 2048
D = 1024
H = 16
HD = 64
HLOC = 4          # heads per core
NPAIR = 2         # head pairs per core
N_CORES = 8
GROUP = 4         # cores per batch (replica group size)
IB = 512          # query block width (matmul moving dim)
JT = 128          # key tile (psum partition dim)
OC = D // GROUP   # output-projection column shard per core (256)
SCALE = 1.0 / 8.0  # 1/sqrt(hd)


def _build_bass(s=S):
    """Build the SPMD Bass program (one NeuronCore's view)."""
    import concourse.bacc as bacc
    import concourse.mybir as mybir
    import concourse.tile as tile

    f32 = mybir.dt.float32
    f32r = mybir.dt.float32r
    bf16 = mybir.dt.bfloat16
    Alu = mybir.AluOpType
    Act = mybir.ActivationFunctionType

    n_ib = s // IB           # query blocks
    n_st = s // 128          # 128-row sequence tiles
    n_dt = D // 128          # contraction tiles for D

    # Bacc (not plain Bass): its compile() lowers multi-wait sync_infos into
    # event-semaphore nops, which walrus codegen requires.
    nc = bacc.Bacc(num_devices=N_CORES)

    xt = nc.declare_dram_parameter("xt", [D, s], bf16, isOutput=False)
    wqk = nc.declare_dram_parameter("wqk", [D, 512], bf16, isOutput=False)
    wv = nc.declare_dram_parameter("wv", [D, 256], bf16, isOutput=False)
    bqk = nc.declare_dram_parameter("bqk", [128, 4], f32, isOutput=False)
    bv = nc.declare_dram_parameter("bv", [128, 256], f32, isOutput=False)
    onec = nc.declare_dram_parameter("onec", [128, 256], f32, isOutput=False)
    wp = nc.declare_dram_parameter("wp", [D, OC], bf16, isOutput=False)
    bp = nc.declare_dram_parameter("bp", [128, 2], f32, isOutput=False)
    msk = nc.declare_dram_parameter("msk", [128, 128], f32, isOutput=False)
    # y is TRANSPOSED [oc, s]: the projection then runs as 4 long psum
    # accumulation chains (moving dim 1024) instead of 128 short matmuls
    y = nc.declare_dram_parameter("y", [OC, s], f32, isOutput=True)

    with tile.TileContext(nc) as tc:
        with (
            tc.tile_pool(name="const", bufs=1) as const,
            tc.tile_pool(name="persist", bufs=1) as persist,
            tc.tile_pool(name="dram", bufs=1, space="DRAM") as dram,
        ):
            # ---- allocations (emit nothing) ----
            bqk_sb = const.tile([128, 4], f32)
            bv_sb = const.tile([128, 256], f32)
            onec_sb = const.tile([128, 256], f32)
            bp_sb = const.tile([128, 2], f32)
            msk_sb = const.tile([128, 128], f32)
            wp_sb = const.tile([128, n_dt, OC], bf16)
            warm_sb = const.tile([1, 1], f32)

            xt_sb = persist.tile([128, n_dt, s], bf16)
            wqk_sb = persist.tile([128, n_dt, 512], bf16)
            wv_sb = persist.tile([128, n_dt, 256], bf16)
            qT_sb = persist.tile([128, NPAIR, s], f32r)   # [pair_ch, pair, s]
            kT_sb = persist.tile([128, NPAIR, s], f32r)
            # [:, st, h, 0:64] = v channels, [:, st, h, 64:128] = 1.0
            v_sb = persist.tile([128, n_st, HLOC, 128], f32r)
            aT_sb = persist.tile([128, NPAIR, s], bf16)
            agf_sb = persist.tile([128, 2 * GROUP, s], bf16)  # gathered rows

            # collective buffers: [0] = pair-0 full-s, [2]/[3] = pair-1
            # s-halves (split so the first half gathers under compute)
            ag_widths = {0: s, 2: s // 2, 3: s // 2}
            ag_in = {
                i: dram.tile([128, w], bf16, name=f"ag_in{i}")
                for i, w in ag_widths.items()
            }
            ag_out = {
                i: dram.tile([GROUP * 128, w], bf16, name=f"ag_out{i}")
                for i, w in ag_widths.items()
            }

            # ---- DMA emission, critical-path order ----
            # 1. qkv weights + first quarter of xT: gates the first matmuls.
            nc.sync.dma_start(
                out=wqk_sb, in_=wqk.rearrange("(t p) c -> p t c", p=128)
            )
            nc.sync.dma_start(
                out=wv_sb, in_=wv.rearrange("(t p) c -> p t c", p=128)
            )
            # one trigger per xt quarter: each dma_start costs ~0.7us on the
            # serial Sync queue, so chunk count — not bytes — gated startup
            nc.sync.dma_start(
                out=xt_sb[:, :, 0 : s // 4],
                in_=xt.rearrange("(t p) ss -> p t ss", p=128)[:, :, 0 : s // 4],
            )
            # 2. small consts the first attention block needs.
            nc.sync.dma_start(out=bqk_sb, in_=bqk[:, :])
            nc.sync.dma_start(out=bv_sb, in_=bv[:, :])
            nc.sync.dma_start(out=onec_sb, in_=onec[:, :])
            nc.sync.dma_start(out=msk_sb, in_=msk[:, :])
            # dummy exp: pulls the ACT exp table load off the critical path
            nc.scalar.activation(
                out=warm_sb, in_=bqk_sb[0:1, 0:1], func=Act.Exp, scale=0.0
            )
            # 3. rest of xT + projection consts stream under compute.
            for q in range(1, 4):
                nc.sync.dma_start(
                    out=xt_sb[:, :, q * s // 4 : (q + 1) * s // 4],
                    in_=xt.rearrange("(t p) ss -> p t ss", p=128)[
                        :, :, q * s // 4 : (q + 1) * s // 4
                    ],
                )
            nc.sync.dma_start(out=bp_sb, in_=bp[:, :])
            nc.sync.dma_start(
                out=wp_sb, in_=wp.rearrange("(t p) c -> p t c", p=128)
            )

            # ---- phase 1 + 2: QKV projection interleaved with attention ----
            def v_for(st_lo, st_hi, pool):
                # v natural: lhsT = xT tile [d, s-tile], rhs = Wv [d, 256]
                for st in range(st_lo, st_hi):
                    psv = pool.tile([128, 256], f32, name="psv", tag="pss")
                    for dt in range(n_dt):
                        nc.tensor.matmul(
                            psv,
                            lhsT=(xt_sb[:, dt, st * 128 : (st + 1) * 128]),
                            rhs=(wv_sb[:, dt, :]),
                            start=(dt == 0),
                            stop=(dt == n_dt - 1),
                        )
                    nc.vector.tensor_tensor(
                        out=v_sb[:, st, :, 0:64],
                        in0=psv.rearrange("p (h e) -> p h e", h=HLOC),
                        in1=bv_sb.rearrange("p (h e) -> p h e", h=HLOC),
                        op=Alu.add,
                    )
                    # ones columns 64..127: AV replicates the softmax
                    # denominator across psum rows 64..127
                    nc.vector.tensor_copy(
                        out=v_sb[:, st, :, 64:128],
                        in_=onec_sb.rearrange("p (h e) -> p h e", h=HLOC),
                    )

            def qkT_sb(t, sb, pool):
                # qT/kT: lhsT = W tile [d,c], rhs = xT [d, s-block]
                # c-tile t: 0,1 = q pair0/1; 2,3 = k pair0/1
                ps = pool.tile([128, IB], f32, name="ps", tag="pss")
                for dt in range(n_dt):
                    nc.tensor.matmul(
                        ps,
                        lhsT=(wqk_sb[:, dt, t * 128 : (t + 1) * 128]),
                        rhs=(xt_sb[:, dt, sb * IB : (sb + 1) * IB]),
                        start=(dt == 0),
                        stop=(dt == n_dt - 1),
                    )
                dst = qT_sb if t < 2 else kT_sb
                nc.vector.tensor_scalar_add(
                    out=dst[:, t % 2, sb * IB : (sb + 1) * IB],
                    in0=ps,
                    scalar1=bqk_sb[:, t : t + 1],
                )

            with (
                tc.tile_pool(name="ps_s", bufs=2, space="PSUM") as ps_s,
                tc.tile_pool(name="ps_av", bufs=2, space="PSUM") as ps_av,
                tc.tile_pool(name="pt", bufs=4) as ptpool,
                tc.tile_pool(name="small", bufs=4) as small,
            ):

                def scores_av_emit(pair, ib):
                    """Scores + exp + causal mask + AV for query block ib.
                    Returns the two [128,IB] psum accumulators (rows 64..127
                    = replicated softmax denominator). Diagonal key tile k
                    only touches query columns >= 128k (the rest are fully
                    masked), which shortens its score/exp/AV moving dim."""
                    njt = 4 * (ib + 1)  # key tiles needed (j <= i)
                    avs = [
                        ps_av.tile([128, IB], f32, name=f"av{hh}", tag=f"av{hh}")
                        for hh in range(2)
                    ]
                    # diagonal key tiles first: their mask multiply then
                    # overlaps the long non-diagonal score/AV stream
                    jt_order = list(range(4 * ib, njt)) + list(range(4 * ib))

                    def av_emit(pt, jt, lo, jseq):
                        for hh in range(2):
                            nc.tensor.matmul(
                                avs[hh][:, lo:IB],
                                lhsT=(v_sb[:, jt, pair * 2 + hh, :]),
                                rhs=(pt[:, hh, lo:IB]),
                                start=(jseq == 0),
                                stop=(jseq == njt - 1),
                            )

                    # AV(jt) is emitted AFTER scores(jt+1): the exp latency
                    # then hides behind the next tile's score matmuls instead
                    # of head-of-line blocking the PE queue
                    prev_av = None
                    for jseq, jt in enumerate(jt_order):
                        k = jt - 4 * ib  # >= 0: diagonal tile index
                        lo = 128 * k if k > 0 else 0  # first live query col
                        pss = ps_s.tile([128, 2 * IB], f32, name="pss", tag="pss")
                        for hh in range(2):
                            off = hh * 64
                            nc.tensor.matmul(
                                pss[:, hh * IB + lo : (hh + 1) * IB],
                                lhsT=(kT_sb[
                                        off : off + 64,
                                        pair,
                                        jt * 128 : (jt + 1) * 128,
                                    ]
                                ),
                                rhs=(qT_sb[
                                        off : off + 64,
                                        pair,
                                        ib * IB + lo : (ib + 1) * IB,
                                    ]
                                ),
                                start=True,
                                stop=True,
                                tile_position=(off, 0),
                            )
                        if prev_av is not None:
                            av_emit(*prev_av)
                        pt = ptpool.tile([128, 2, IB], f32r, name="pt")
                        nc.scalar.activation(
                            out=pt[:, :, lo:IB],
                            in_=pss.rearrange("p (h q) -> p h q", h=2)[
                                :, :, lo:IB
                            ],
                            func=Act.Exp,
                            scale=SCALE,
                        )
                        for hh in range(2):
                            if k >= 0:  # causal triangle on the k-th block
                                eng = nc.vector if hh == 0 else nc.gpsimd
                                eng.tensor_tensor(
                                    out=pt[:, hh, lo : lo + 128],
                                    in0=pt[:, hh, lo : lo + 128],
                                    in1=msk_sb,
                                    op=Alu.mult,
                                )
                        prev_av = (pt, jt, lo, jseq)
                    av_emit(*prev_av)
                    return avs

                def normalize_emit(pair, ib, avs):
                    """aT[:, ib block] = av[0:64] * (1 / av[64:128]): the
                    denominator arrives already replicated from the ones
                    columns of v, so this is one fast-approx reciprocal
                    (~18 bits, 5x faster than exact) + one multiply per
                    head, both DVE, no PE involvement."""
                    for hh in range(2):
                        zsb = small.tile([64, IB], f32, name="zsb")
                        nc.vector.tensor_copy(out=zsb, in_=avs[hh][64:128, :])
                        recr = small.tile([64, IB], f32, name="recr")
                        nc.vector.reciprocal_approx_fast(out=recr, in_=zsb)
                        nc.vector.tensor_tensor(
                            out=aT_sb[
                                hh * 64 : (hh + 1) * 64,
                                pair,
                                ib * IB : (ib + 1) * IB,
                            ],
                            in0=recr,
                            in1=avs[hh][0:64, :],
                            op=Alu.mult,
                        )

                def gather_emit(pair, half):
                    """Stage + AllGather + reload one s-half of a pair's aT.
                    half=None gathers the full s range in one collective."""
                    lo, hi = (0, s) if half is None else (
                        half * s // 2, (half + 1) * s // 2
                    )
                    idx = 2 * pair + (half or 0)
                    gin, gout = ag_in[idx], ag_out[idx]
                    nc.sync.dma_start(out=gin, in_=aT_sb[:, pair, lo:hi])
                    nc.gpsimd.collective_compute(
                        "AllGather",
                        Alu.bypass,
                        replica_groups=[[0, 1, 2, 3], [4, 5, 6, 7]],
                        ins=[gin[:, :]],
                        outs=[gout[:, :]],
                    )
                    # reload by s-quarter (all 4 gathered row-blocks per
                    # trigger): the first projection quarter that needs this
                    # gather can start after ~1MB instead of the full reload
                    w = hi - lo
                    for q in range(2):
                        nc.sync.dma_start(
                            out=agf_sb[
                                :,
                                pair * GROUP : (pair + 1) * GROUP,
                                lo + q * w // 2 : lo + (q + 1) * w // 2,
                            ],
                            in_=gout.rearrange("(t p) ss -> p t ss", p=128)[
                                :, :, q * w // 2 : (q + 1) * w // 2
                            ],
                        )

                for pair in range(NPAIR):
                    pending = None
                    for ib in range(n_ib):
                        # just-in-time qT/kT for this query block: the first
                        # score matmuls start after one xT quarter arrives
                        qkT_sb(pair, ib, ps_s)
                        qkT_sb(2 + pair, ib, ps_s)
                        if pair == 0:
                            # v quarter-blocks on demand: attention for this
                            # ib only needs v key-tiles st <= 4*ib+3
                            v_for(4 * ib, 4 * ib + 4, ps_s)
                        avs = scores_av_emit(pair, ib)
                        # deferred normalize: ib-1's reciprocal chain hides
                        # under ib's score/AV stream on the PE
                        if pending is not None:
                            normalize_emit(pair, ib - 1, pending)
                        pending = avs
                        if pair == 1 and ib == 2:
                            # aT[:, 1, 0:s/2] (ibs 0-1) is complete: gather
                            # the first half under ibs 2-3's compute
                            gather_emit(1, 0)
                    normalize_emit(pair, n_ib - 1, pending)
                    if pair == 0:
                        # single collective: fully hidden under pair 1
                        gather_emit(0, None)
                    else:
                        gather_emit(1, 1)

            # ---- phase 3: output projection, transposed + split by s ----
            # yT[oc, s] = wp^T @ a_gathered: 4 psum chains of 8 matmuls with
            # moving dim s/2 = 1024 (dense, p-state friendly). The s-half 0
            # chains need only AG0 + AG1a and run while AG1b flies.
            with (
                tc.tile_pool(name="ps_y", bufs=4, space="PSUM") as ps_y,
                tc.tile_pool(name="yout", bufs=3) as yout,
            ):
                for sq in range(4):  # s quarters (psum bank = 512 f32)
                    lo, hi = sq * s // 4, (sq + 1) * s // 4
                    yts = {}
                    for ocb in range(2):
                        yts[ocb] = ps_y.tile([128, s // 4], f32, name="yt")
                    for t in range(2 * GROUP):
                        for ocb in range(2):
                            nc.tensor.matmul(
                                yts[ocb],
                                lhsT=(wp_sb[:, t, ocb * 128 : (ocb + 1) * 128]),
                                rhs=(agf_sb[:, t, lo:hi]),
                                start=(t == 0),
                                stop=(t == 2 * GROUP - 1),
                            )
                    for ocb in range(2):
                        ysb = yout.tile([128, s // 4], f32, name="ysb")
                        nc.vector.tensor_scalar_add(
                            out=ysb,
                            in0=yts[ocb],
                            scalar1=bp_sb[:, ocb : ocb + 1],
                        )
                        nc.sync.dma_start(
                            out=y[ocb * 128 : (ocb + 1) * 128, lo:hi],
                            in_=ysb,
                        )

    nc.compile()
    return nc


def _shard_inputs(x, w_attn, b_attn, w_proj, b_proj, s=S):
    """Host-side sharding: build the per-core input maps."""
    import ml_dtypes

    bfl = ml_dtypes.bfloat16
    x = np.asarray(x, dtype=np.float32)
    w_attn = np.asarray(w_attn, dtype=np.float32)
    b_attn = np.asarray(b_attn, dtype=np.float32)
    w_proj = np.asarray(w_proj, dtype=np.float32)
    b_proj = np.asarray(b_proj, dtype=np.float32)

    # causal triangle tile: msk[j, i] = 1.0 if i >= j (shared by every
    # diagonal key tile after its fully-masked columns are skipped)
    msk = (np.arange(128)[None, :] >= np.arange(128)[:, None]).astype(
        np.float32
    )

    # w_proj row permutation matching the two per-pair gathers:
    # half 0 = [core0 pair0 (128 rows), core1 pair0, ...], half 1 = pair1.
    perm = np.concatenate(
        [np.arange(c * 256, c * 256 + 128) for c in range(GROUP)]
        + [np.arange(c * 256 + 128, (c + 1) * 256) for c in range(GROUP)]
    )

    in_maps = []
    for core in range(N_CORES):
        b, g = divmod(core, GROUP)
        hs = list(range(g * HLOC, (g + 1) * HLOC))
        xt = np.ascontiguousarray(x[b].T).astype(bfl)
        qcols = np.concatenate(
            [w_attn[:, h * HD : (h + 1) * HD] for h in hs], axis=1
        )
        kcols = np.concatenate(
            [w_attn[:, D + h * HD : D + (h + 1) * HD] for h in hs], axis=1
        )
        vcols = np.concatenate(
            [w_attn[:, 2 * D + h * HD : 2 * D + (h + 1) * HD] for h in hs],
            axis=1,
        )
        wqk = np.concatenate([qcols, kcols], axis=1).astype(bfl)
        bq = np.concatenate([b_attn[h * HD : (h + 1) * HD] for h in hs])
        bk = np.concatenate([b_attn[D + h * HD : D + (h + 1) * HD] for h in hs])
        bvv = np.concatenate(
            [b_attn[2 * D + h * HD : 2 * D + (h + 1) * HD] for h in hs]
        )
        bqk = np.concatenate([bq, bk]).reshape(4, 128).T.copy()  # [128, 4]
        bv = np.broadcast_to(bvv, (128, 256)).copy()
        wpc = np.ascontiguousarray(
            w_proj[perm][:, g * OC : (g + 1) * OC]
        ).astype(bfl)
        # transposed-projection bias: column ocb holds b_proj for oc rows
        # [ocb*128, (ocb+1)*128) of this core's shard
        bpc = b_proj[g * OC : (g + 1) * OC].reshape(2, 128).T.copy()
        in_maps.append(
            dict(
                xt=xt, wqk=wqk, wv=vcols.astype(bfl), bqk=bqk, bv=bv,
                onec=np.ones((128, 256), np.float32), wp=wpc, bp=bpc, msk=msk,
            )
        )
    return in_maps


def _unshard(results):
    y = np.empty((B, S, D), np.float32)
    for core in range(N_CORES):
        b, g = divmod(core, GROUP)
        y[b, :, g * OC : (g + 1) * OC] = results[core]["y"].T
    return y


_NC_CACHE = {}


def kernel(x, w_attn, b_attn, w_proj, b_proj):
    from concourse.bass_utils import run_bass_kernel_spmd

    if S not in _NC_CACHE:
        _NC_CACHE[S] = _build_bass(S)
    nc = _NC_CACHE[S]
    in_maps = _shard_inputs(x, w_attn, b_attn, w_proj, b_proj)
    res = run_bass_kernel_spmd(nc, in_maps, list(range(N_CORES)))
    return _unshard(res.results)
